# revision 5
# baseline (speedup 1.0000x reference)
"""Multi-head self-attention (B=8, S=1024, E=768, H=12, D=64) on 8 NeuronCores.

Sharding: data-parallel over batch — one batch element per core, weights
replicated, no collectives.  v2 (XBAR-transpose + ACT-normalization
rework); v1 measured ~233us HW exec, rel-RMS err ~1.6e-3.

Measured facts driving the design (NTFF profiles + probe kernels):
 - PE stream: fp16 moving operands cross the array at 1 cycle/row
   (~215ns for N=512 @2.4GHz); fp32 at 2 cycles/row.  All matmul
   operands are fp16.  Per-MM marginal cost measured ~257ns (N=512):
   ~40ns of exposed LDWEIGHTS per MM (walrus can't dedup or hide them
   fully; --enable-ldw-opt rejects pre-legalized LDWEIGHTS).
 - PE busy (v1) was 189us of the 239.7us span: the engine-floor.  The
   48 PE transposes (~430ns each fp32) are replaced in v2 by XBAR DMA
   transposes (InstDmaTransposeAnt, 2-byte dtypes, 16x128 tiles at
   ~14ns/tile): x is DVE-cast f32->f16, then dma_start_transpose fills
   xT[p, k, s] = x[s, k*128+p] (3D-out ordering verified on HW).
   Frees ~10us of PE and ~7us of DVE evacuations.
 - ACT exp is the other floor: 12.6M exps at ~1 col/cycle/partition
   @0.96GHz = ~107us.  Softmax normalization moved to ACT as
   rec = Exp(-Ln(sums)) (measured rel err ~9e-6 RMS, better than the
   v1 Newton bit-trick's 2.6e-3); ACT reads PSUM at a different
   partition base than its output correctly (verified on HW), so the
   sums rows land directly in the swapped partition halves and the v1
   half-swap SBUF DMAs are gone.  DVE keeps only 2 tensor_muls per
   (pair, q-half).  DVE tensor ops must stay partition-aligned (walrus
   verifier NCC_IBIR297) and DVE PSUM reads partition-aligned with the
   output (measured silent corruption otherwise).
 - Queue layout at startup: x DMAs on sync, Wv staging DMAs on the
   otherwise-idle gpsimd queue (interleaved with the V_ext ones
   memsets), f32->f16 casts on DVE, XBAR transposes on the scalar
   queue (hwdge = {SP, Activation} only).
 - Phase 3 pipelining: Q/K projections for pair m+1 are emitted one
   per (pair m, q-half) unit so PE has independent work while ACT
   drains the exps; the output projection for s-tiles 0-3 (which only
   need q-half 0 of concatT) is emitted inside the last pair's q2=0
   unit, leaving only s-tiles 4-7 plus one normalization for the tail.

Per-core dataflow:
  1. x tiles DMA in; DVE casts to fp16; XBAR DMA-transposes write
     xT[e, s] directly (no PE, no PSUM).
  2. V scattered into V_ext[s, ktile, head, 128] = [V_h+bv | ones]
     (even head) or [ones | V_h+bv] (odd head); ones-halves memset
     strided on gpsimd; adding bv here is exact (softmax rows sum to 1).
  3. QT/KT per head pair: lhsT=W tile, rhs=xT; bias via
     tensor_scalar_add on the PSUM evacuation (fp16 out).
  4. Per (pair, q-half): scores^T[k,q] = KT.T @ QT (row halves at
     partition base 0/64), exp on ACT with the 1/sqrt(D)=1/8 scale
     folded in (scores ~ N(0,1), no max guard needed).
  5. attnV: one M=128 matmul per (head, ktile) -> rows [attn^T|sums]
     (even) / [sums|attn^T] (odd).  Normalization: ACT Ln of the sums
     rows (cross-base into the attn rows' partitions), one ACT
     Exp(-x), two DVE tensor_muls into concatT.
  6. out = concatT.T @ Wo + bo (bo broadcast via partition-step-0 DMA).

Workarounds baked in: this walrus build rejects instructions carrying
more than ~1-2 sync waits; _split_excess_waits and the patched
TileContext tail hoist surplus waits onto standalone EVSEM ops.
InstDmaTransposeAnt can carry no waits at all (codegen "Too many sync
wait commands"), so all of its waits are hoisted.
"""
import sys
sys.path.insert(0, "/opt/trn_rl_repo")
from contextlib import ExitStack

import numpy as np

import concourse.bass as bass
import concourse.bass_utils as _bu
import concourse.tile as tile
from concourse import mybir
from concourse.bass_utils import run_bass_kernel_spmd
from concourse.vector_clock import ScopedClock


def _split_drain_and_barrier(self, tick_clock, wait_clock):
    """TileContext tail with the final drain's waits split one-per-instruction."""
    drain_inst = self.nc.sync.drain()
    wait_clock.add_sem_waits(
        drain_inst.ins, ScopedClock({None: tick_clock.global_clock})
    )
    si = drain_inst.ins.sync_info
    waits = list(si.on_wait) if si is not None and si.on_wait else []
    if len(waits) > 1:
        si.on_wait = []
        by_num = {h.num: h for h in self.sems.allocated().values()}
        for w in waits:
            self.nc.sync.wait_ge(by_num[w.id], w.wait_value)
    self.nc.all_engine_barrier()
    popped = self.nc._tile_sem_poison_stack.pop()
    assert popped is self._sem_poison
    self.nc.clear_and_free_semaphores(list(self.sems.allocated().values()))
    self.nc.all_engine_barrier()


tile.TileContext._drain_and_barrier = _split_drain_and_barrier


def _split_excess_waits(nc):
    """Hoist excess per-instruction sync waits into standalone EVSEM waits.

    InstDmaTransposeAnt cannot encode ANY sync wait (walrus codegen
    "Too many sync wait commands"), so all of its waits move onto
    EVSEMs before it; other instructions keep 1 wait (EVSEM: 2)."""
    counter = 0
    for f in nc.m.functions:
        for bb in f.blocks:
            insts = bb.instructions
            out = []
            for inst in insts:
                si = inst.sync_info
                tname = type(inst).__name__
                if "DmaTranspose" in tname:
                    cap = 0
                elif isinstance(inst, mybir.InstEventSemaphore):
                    cap = 2
                else:
                    cap = 1
                if si is not None and si.on_wait and len(si.on_wait) > cap:
                    waits = list(si.on_wait)
                    for w in waits[cap:]:
                        counter += 1
                        ev = mybir.InstEventSemaphore(name=f"I-wsplit-{counter}")
                        ev.engine = inst.engine
                        ev.sync_info = mybir.SyncInfo(on_wait=[w], on_update=[])
                        out.append(ev)
                    si.on_wait = waits[:cap]
                out.append(inst)
            if len(out) != len(insts):
                insts[:] = out
    return counter


P = 128
S = 1024
E = 768
H = 12
D = 64
KT = E // P        # 6 e-tiles
ST = S // P        # 8 s-tiles
NPAIR = H // 2     # 6 head pairs
QTILE = 512
NQ = S // QTILE    # 2 q-tiles
ESLICES = [(0, 512), (512, 256)]

f32 = mybir.dt.float32
f16 = mybir.dt.float16
bf16 = mybir.dt.bfloat16
EXP = mybir.ActivationFunctionType.Exp
LN = mybir.ActivationFunctionType.Ln

_NC_CACHE = {}


def build(mm_dtype="f16", e_dtype="f16"):
    mdt = {"f16": f16, "bf16": bf16}[mm_dtype]
    edt = {"f16": f16, "bf16": bf16}[e_dtype]
    nc = bass.Bass()
    x_d = nc.declare_dram_parameter("x", [S, E], f32, isOutput=False)
    Wq_d = nc.declare_dram_parameter("Wq", [E, E], f32, isOutput=False)
    Wk_d = nc.declare_dram_parameter("Wk", [E, E], f32, isOutput=False)
    Wv_d = nc.declare_dram_parameter("Wv", [E, E], f32, isOutput=False)
    Wo_d = nc.declare_dram_parameter("Wo", [E, E], f32, isOutput=False)
    bq_d = nc.declare_dram_parameter("bq", [E], f32, isOutput=False)
    bk_d = nc.declare_dram_parameter("bk", [E], f32, isOutput=False)
    bv_d = nc.declare_dram_parameter("bv", [E], f32, isOutput=False)
    bo_d = nc.declare_dram_parameter("bo", [E], f32, isOutput=False)
    out_d = nc.declare_dram_parameter("out", [S, E], f32, isOutput=True)

    with ExitStack() as ctx:
        tc = ctx.enter_context(tile.TileContext(nc))
        singles = ctx.enter_context(tc.tile_pool(name="singles", bufs=1))
        xld = ctx.enter_context(tc.tile_pool(name="xld", bufs=4))
        x16p = ctx.enter_context(tc.tile_pool(name="x16p", bufs=4))
        wst = ctx.enter_context(tc.tile_pool(name="wst", bufs=3))
        wqk = ctx.enter_context(tc.tile_pool(name="wqk", bufs=2))
        wbig = ctx.enter_context(tc.tile_pool(name="wbig", bufs=1))
        qkp = ctx.enter_context(tc.tile_pool(name="qkp", bufs=2))
        ep = ctx.enter_context(tc.tile_pool(name="ep", bufs=2))
        np_pool = ctx.enter_context(tc.tile_pool(name="norm", bufs=2))
        outp = ctx.enter_context(tc.tile_pool(name="outp", bufs=2))
        bcast = ctx.enter_context(tc.tile_pool(name="bcast", bufs=1))
        # PSUM: S ([P,2,512]x2 = 4 banks) + mm ([P,512]x2) + att ([P,512]x2)
        psum = ctx.enter_context(tc.tile_pool(name="psum", bufs=2, space="PSUM"))

        # ---- persistent big buffers ----
        xT = singles.tile([P, KT, S], mdt)          # x^T  [e_in, s]
        V_ext = singles.tile([P, ST, H, P], edt)    # [s, ktile, head, ...]
        concatT = singles.tile([P, NPAIR, S], mdt)  # attn^T by pair

        # ---- phase 0: DMAs + casts + XBAR transposes ----
        # sync queue: x tiles (+ b vectors); gpsimd: Wv staging, bcasts,
        # V_ext ones-memsets; DVE: all f32->f16 casts; scalar: transposes.
        x_sb, x16 = {}, {}
        for st in range(ST):
            x_sb[st] = xld.tile([P, E], f32, tag="x", name="x_sb")
            x16[st] = x16p.tile([P, E], mdt, tag="x16", name="x16")

        Wv_sb = wbig.tile([P, KT, E], mdt, tag="wbig")
        Wv_re = Wv_d[:].rearrange("(ko p) m -> p ko m", p=P)
        wv_stg = []
        for j in range(KT):
            wv_stg.append(wst.tile([P, E], f32, tag="wstage", name="wstage"))

        def bcast_load(dst, src_ap):  # [E] -> [P, E] partition-step-0 DMA
            nc.gpsimd.dma_start(
                out=dst,
                in_=bass.AP(tensor=src_ap.tensor, offset=src_ap.offset,
                            ap=[[0, P]] + [list(a) for a in src_ap.ap]))

        bv_bc = bcast.tile([P, E], f32, tag="bvbc")
        bo_bc = bcast.tile([P, E], f32, tag="bobc")
        v4 = V_ext[:].rearrange("p st (hh two) d -> p st hh two d", two=2)

        # sync queue: x DMAs in order
        for st in range(ST):
            nc.sync.dma_start(x_sb[st][:], x_d[st * P:(st + 1) * P, :])
        # gpsimd queue: Wv staging interleaved with bcasts + ones-memsets
        nc.gpsimd.dma_start(wv_stg[0][:], Wv_re[:, 0, :])
        nc.gpsimd.dma_start(wv_stg[1][:], Wv_re[:, 1, :])
        bcast_load(bv_bc[:], bv_d[:])
        nc.gpsimd.memset(v4[:, 0, :, 0, D:P], 1.0)
        nc.gpsimd.memset(v4[:, 0, :, 1, 0:D], 1.0)
        for j in range(2, KT):
            nc.gpsimd.dma_start(wv_stg[j][:], Wv_re[:, j, :])
            st = j - 1
            nc.gpsimd.memset(v4[:, st, :, 0, D:P], 1.0)
            nc.gpsimd.memset(v4[:, st, :, 1, 0:D], 1.0)
        for st in range(KT - 1, ST):
            nc.gpsimd.memset(v4[:, st, :, 0, D:P], 1.0)
            nc.gpsimd.memset(v4[:, st, :, 1, 0:D], 1.0)
        bcast_load(bo_bc[:], bo_d[:])
        # DVE: casts — x0, Wv0, Wv1, x1, Wv2, x2, Wv3, x3, Wv4, x4, Wv5, x5-7
        dve_order = [("x", 0), ("w", 0), ("w", 1), ("x", 1), ("w", 2),
                     ("x", 2), ("w", 3), ("x", 3), ("w", 4), ("x", 4),
                     ("w", 5), ("x", 5), ("x", 6), ("x", 7)]
        for kind, i in dve_order:
            if kind == "x":
                nc.vector.tensor_copy(x16[i][:], x_sb[i][:])
            else:
                nc.vector.tensor_copy(Wv_sb[:, i, :], wv_stg[i][:])
        # scalar queue: XBAR transposes into xT
        for st in range(ST):
            nc.scalar.dma_start_transpose(
                xT[:, :, st * P:(st + 1) * P], x16[st][:])
        # small bias loads (sync, after x)
        bq_sb = singles.tile([P, KT], f32)
        bk_sb = singles.tile([P, KT], f32)
        nc.sync.dma_start(bq_sb[:], bq_d[:].rearrange("(o p) -> p o", p=P))
        nc.sync.dma_start(bk_sb[:], bk_d[:].rearrange("(o p) -> p o", p=P))

        # ---- phase 2: V projection per s-tile ----
        def vproj_st(st):
            pv = psum.tile([P, 2, 512], f32, tag="S", name="pv")
            for k in range(KT):  # k-outer: xT stationary reused across nsi
                for nsi, (noff, nsz) in enumerate(ESLICES):
                    nc.tensor.matmul(
                        pv[:, nsi, :nsz],
                        xT[:, k, st * P:(st + 1) * P],
                        Wv_sb[:, k, noff:noff + nsz],
                        start=(k == 0), stop=(k == KT - 1),
                    )
            # batched scatter: evens -> [V|ones] cols 0:64, odds -> 64:128
            for nsi, (noff, nsz) in enumerate(ESLICES):
                nh = nsz // P
                hh0 = 4 * nsi
                pvr = pv[:, nsi, :nsz].rearrange(
                    "p (hh two d) -> p hh two d", two=2, d=D)
                bvr = bv_bc[:, noff:noff + nsz].rearrange(
                    "p (hh two d) -> p hh two d", two=2, d=D)
                nc.vector.tensor_add(
                    v4[:, st, hh0:hh0 + nh, 0, 0:D], pvr[:, :, 0, :],
                    bvr[:, :, 0, :])
                nc.vector.tensor_add(
                    v4[:, st, hh0:hh0 + nh, 1, D:P], pvr[:, :, 1, :],
                    bvr[:, :, 1, :])

        for st in range(ST):
            vproj_st(st)

        # ---- phase 3: head pairs, software-pipelined ----
        wq_t, wk_t, qt_t, kt_t = {}, {}, {}, {}
        Wq_re = Wq_d[:].rearrange("(ko p) m -> p ko m", p=P)
        Wk_re = Wk_d[:].rearrange("(ko p) m -> p ko m", p=P)

        def load_w(m):
            wq_t[m] = wqk.tile([P, KT, P], mdt, tag="wq", name="wq_m")
            wk_t[m] = wqk.tile([P, KT, P], mdt, tag="wk", name="wk_m")
            for which, dst, src in (("q", wq_t[m], Wq_re), ("k", wk_t[m], Wk_re)):
                stg = wst.tile([P, KT, P], f32, tag="wqs" + which, name="wqs")
                nc.sync.dma_start(stg[:], src[:, :, m * P:(m + 1) * P])
                nc.vector.tensor_copy(dst[:], stg[:])

        def proj_one(m, which):
            """12 matmuls: full QT_m (or KT_m) over both q-halves."""
            w = wq_t[m] if which == "q" else wk_t[m]
            bias = bq_sb if which == "q" else bk_sb
            t = qkp.tile([P, S], mdt, tag=which + "t", name=which + "t")
            (qt_t if which == "q" else kt_t)[m] = t
            for q2 in range(NQ):
                qsl = slice(q2 * QTILE, (q2 + 1) * QTILE)
                pq = psum.tile([P, 512], f32, tag="mm", name="pq")
                for k in range(KT):
                    nc.tensor.matmul(pq[:], w[:, k, :], xT[:, k, qsl],
                                     start=(k == 0), stop=(k == KT - 1))
                nc.vector.tensor_scalar_add(t[:, qsl], pq[:], bias[:, m:m + 1])

        def wload_big(dst16, src_re):
            for j in range(KT):
                stg = wst.tile([P, E], f32, tag="wstage", name="wstage")
                nc.sync.dma_start(stg[:], src_re[:, j, :])
                nc.vector.tensor_copy(dst16[:, j, :], stg[:])

        def outproj_st(st):
            o_sb = outp.tile([P, E], f32, tag="o")
            for nsi, (noff, nsz) in enumerate(ESLICES):
                po = psum.tile([P, 512], f32, tag="mm", name="po")
                for k in range(KT):
                    nc.tensor.matmul(
                        po[:, :nsz],
                        concatT[:, k, st * P:(st + 1) * P],
                        Wo_sb[:, k, noff:noff + nsz],
                        start=(k == 0), stop=(k == KT - 1),
                    )
                nc.vector.tensor_add(o_sb[:, noff:noff + nsz], po[:, :nsz],
                                     bo_bc[:, noff:noff + nsz])
            nc.sync.dma_start(out_d[st * P:(st + 1) * P, :], o_sb[:])

        load_w(0)
        proj_one(0, "q")
        proj_one(0, "k")
        # Wo loaded+cast here: overlaps the attention phase; the wbig slot
        # becomes free once the last V-proj matmul has read Wv.
        Wo_sb = wbig.tile([P, KT, E], mdt, tag="wbig")
        wload_big(Wo_sb, Wo_d[:].rearrange("(ko p) m -> p ko m", p=P))
        for m in range(NPAIR):
            if m + 1 < NPAIR:
                load_w(m + 1)
            qt_m, kt_m = qt_t[m], kt_t[m]
            for q2 in range(NQ):
                qsl = slice(q2 * QTILE, (q2 + 1) * QTILE)
                # proj for the next pair FIRST: independent PE work in front
                # of the scores c-loop, which is paced by ACT draining the
                # previous unit's exps out of the S-ring.
                if m + 1 < NPAIR:
                    proj_one(m + 1, "q" if q2 == 0 else "k")
                e_a = ep.tile([P, ST, QTILE], edt, tag="eA")
                e_b = ep.tile([P, ST, QTILE], edt, tag="eB")
                for c in range(ST // 2):
                    s_a = psum.tile([P, 2, 512], f32, tag="S", name="s_a")
                    s_b = psum.tile([P, 2, 512], f32, tag="S", name="s_b")
                    for kk in range(2):
                        ktile = c * 2 + kk
                        ksl = slice(ktile * P, (ktile + 1) * P)
                        nc.tensor.matmul(s_a[:, kk, :], kt_m[0:D, ksl],
                                         qt_m[0:D, qsl], start=True, stop=True)
                        nc.tensor.matmul(s_b[:, kk, :], kt_m[D:P, ksl],
                                         qt_m[D:P, qsl], start=True, stop=True)
                    nc.scalar.activation(e_a[:, c * 2:c * 2 + 2, :], s_a[:], EXP, scale=0.125)
                    nc.scalar.activation(e_b[:, c * 2:c * 2 + 2, :], s_b[:], EXP, scale=0.125)
                # outproj s-tiles 0-3 need only q-half 0 of concatT: slot
                # their matmuls between the last unit's scores (which feed
                # ACT) and its attnV (which waits on those exps).
                if m == NPAIR - 1 and q2 == 1:
                    for st in range(4):
                        outproj_st(st)
                # attnV: rows [attn|sums] (even head) / [sums|attn] (odd head)
                p_a = psum.tile([P, 512], f32, tag="att", name="p_a")
                p_b = psum.tile([P, 512], f32, tag="att", name="p_b")
                for ktile in range(ST):
                    nc.tensor.matmul(p_a[:], V_ext[:, ktile, 2 * m, :],
                                     e_a[:, ktile, :],
                                     start=(ktile == 0), stop=(ktile == ST - 1))
                for ktile in range(ST):
                    nc.tensor.matmul(p_b[:], V_ext[:, ktile, 2 * m + 1, :],
                                     e_b[:, ktile, :],
                                     start=(ktile == 0), stop=(ktile == ST - 1))
                # Normalize: rec = Exp(-Ln(sums)) on ACT; the Ln reads the
                # sums rows cross-base so rec lands in the attn rows'
                # partitions; DVE muls stay fully partition-aligned.
                lnb = np_pool.tile([P, 512], f32, tag="lnb")
                rec = np_pool.tile([P, 512], f32, tag="rec")
                nc.scalar.activation(lnb[0:D, :], p_a[D:P, :], LN)
                nc.scalar.activation(lnb[D:P, :], p_b[0:D, :], LN)
                nc.scalar.activation(rec[:], lnb[:], EXP, scale=-1.0)
                nc.vector.tensor_mul(concatT[0:D, m, qsl], p_a[0:D, :],
                                     rec[0:D, :])
                nc.vector.tensor_mul(concatT[D:P, m, qsl], p_b[D:P, :],
                                     rec[D:P, :])

        # ---- phase 4: output projection, remaining s-tiles ----
        for st in range(4, ST):
            outproj_st(st)

    _split_excess_waits(nc)
    return nc


def run_spmd(inputs, Wq, bq, Wk, bk, Wv, bv, Wo, bo,
             mm_dtype="f16", e_dtype="f16", trace=False):
    key = (mm_dtype, e_dtype)
    if key not in _NC_CACHE:
        _NC_CACHE[key] = build(mm_dtype, e_dtype)
    nc = _NC_CACHE[key]
    x = np.asarray(inputs, dtype=np.float32)
    common = {
        "Wq": np.asarray(Wq, np.float32), "Wk": np.asarray(Wk, np.float32),
        "Wv": np.asarray(Wv, np.float32), "Wo": np.asarray(Wo, np.float32),
        "bq": np.asarray(bq, np.float32), "bk": np.asarray(bk, np.float32),
        "bv": np.asarray(bv, np.float32), "bo": np.asarray(bo, np.float32),
    }
    in_maps = [dict(common, x=np.ascontiguousarray(x[b])) for b in range(x.shape[0])]
    res = run_bass_kernel_spmd(nc, in_maps, core_ids=list(range(len(in_maps))),
                               trace=trace)
    out = np.stack([res.results[b]["out"] for b in range(len(in_maps))], axis=0)
    return out, res


def kernel(inputs, Wq, bq, Wk, bk, Wv, bv, Wo, bo):
    out, _ = run_spmd(inputs, Wq, bq, Wk, bk, Wv, bv, Wo, bo)
    return out


# revision 8
# speedup vs baseline: 1.1060x; 1.1060x over previous
"""Multi-head self-attention (B=8, S=1024, E=768, H=12, D=64) on 8 NeuronCores.

Sharding: data-parallel over batch — one batch element per core, weights
replicated, no collectives.  v2 (XBAR-transpose + ACT-normalization
rework); v1 measured ~233us HW exec, rel-RMS err ~1.6e-3.

Measured facts driving the design (NTFF profiles + probe kernels):
 - PE stream: fp16 moving operands cross the array at 1 cycle/row
   (~215ns for N=512 @2.4GHz); fp32 at 2 cycles/row.  All matmul
   operands are fp16.  Per-MM marginal cost measured ~257ns (N=512):
   ~40ns of exposed LDWEIGHTS per MM (walrus can't dedup or hide them
   fully; --enable-ldw-opt rejects pre-legalized LDWEIGHTS).
 - PE busy (v1) was 189us of the 239.7us span: the engine-floor.  The
   48 PE transposes (~430ns each fp32) are replaced in v2 by XBAR DMA
   transposes (InstDmaTransposeAnt, 2-byte dtypes, 16x128 tiles at
   ~14ns/tile): x is DVE-cast f32->f16, then dma_start_transpose fills
   xT[p, k, s] = x[s, k*128+p] (3D-out ordering verified on HW).
   Frees ~10us of PE and ~7us of DVE evacuations.
 - ACT exp is the other floor: 12.6M exps at ~1 col/cycle/partition
   @0.96GHz = ~107us.  Softmax normalization moved to ACT as
   rec = Exp(-Ln(sums)) (measured rel err ~9e-6 RMS, better than the
   v1 Newton bit-trick's 2.6e-3); ACT reads PSUM at a different
   partition base than its output correctly (verified on HW), so the
   sums rows land directly in the swapped partition halves and the v1
   half-swap SBUF DMAs are gone.  DVE keeps only 2 tensor_muls per
   (pair, q-half).  DVE tensor ops must stay partition-aligned (walrus
   verifier NCC_IBIR297) and DVE PSUM reads partition-aligned with the
   output (measured silent corruption otherwise).
 - Queue layout at startup: x DMAs on sync, Wv staging DMAs on the
   otherwise-idle gpsimd queue (interleaved with the V_ext ones
   memsets), f32->f16 casts on DVE, XBAR transposes on the scalar
   queue (hwdge = {SP, Activation} only).
 - Phase 3 pipelining: Q/K projections for pair m+1 are emitted one
   per (pair m, q-half) unit so PE has independent work while ACT
   drains the exps; the output projection for s-tiles 0-3 (which only
   need q-half 0 of concatT) is emitted inside the last pair's q2=0
   unit, leaving only s-tiles 4-7 plus one normalization for the tail.

Per-core dataflow:
  1. x tiles DMA in; DVE casts to fp16; XBAR DMA-transposes write
     xT[e, s] directly (no PE, no PSUM).
  2. V scattered into V_ext[s, ktile, head, 128] = [V_h+bv | ones]
     (even head) or [ones | V_h+bv] (odd head); ones-halves memset
     strided on gpsimd; adding bv here is exact (softmax rows sum to 1).
  3. QT/KT per head pair: lhsT=W tile, rhs=xT; bias via
     tensor_scalar_add on the PSUM evacuation (fp16 out).
  4. Per (pair, q-half): scores^T[k,q] = KT.T @ QT (row halves at
     partition base 0/64), exp on ACT with the 1/sqrt(D)=1/8 scale
     folded in (scores ~ N(0,1), no max guard needed).
  5. attnV: one M=128 matmul per (head, ktile) -> rows [attn^T|sums]
     (even) / [sums|attn^T] (odd).  Normalization: ACT Ln of the sums
     rows (cross-base into the attn rows' partitions), one ACT
     Exp(-x), two DVE tensor_muls into concatT.
  6. out = concatT.T @ Wo + bo (bo broadcast via partition-step-0 DMA).

Workarounds baked in: this walrus build rejects instructions carrying
more than ~1-2 sync waits; _split_excess_waits and the patched
TileContext tail hoist surplus waits onto standalone EVSEM ops.
InstDmaTransposeAnt can carry no waits at all (codegen "Too many sync
wait commands"), so all of its waits are hoisted.
"""
import sys
sys.path.insert(0, "/opt/trn_rl_repo")
from contextlib import ExitStack

import numpy as np

import concourse.bass as bass
import concourse.bass_utils as _bu
import concourse.tile as tile
from concourse import mybir
from concourse.bass_utils import run_bass_kernel_spmd
from concourse.vector_clock import ScopedClock


def _split_drain_and_barrier(self, tick_clock, wait_clock):
    """TileContext tail with the final drain's waits split one-per-instruction."""
    drain_inst = self.nc.sync.drain()
    wait_clock.add_sem_waits(
        drain_inst.ins, ScopedClock({None: tick_clock.global_clock})
    )
    si = drain_inst.ins.sync_info
    waits = list(si.on_wait) if si is not None and si.on_wait else []
    if len(waits) > 1:
        si.on_wait = []
        by_num = {h.num: h for h in self.sems.allocated().values()}
        for w in waits:
            self.nc.sync.wait_ge(by_num[w.id], w.wait_value)
    self.nc.all_engine_barrier()
    popped = self.nc._tile_sem_poison_stack.pop()
    assert popped is self._sem_poison
    self.nc.clear_and_free_semaphores(list(self.sems.allocated().values()))
    self.nc.all_engine_barrier()


tile.TileContext._drain_and_barrier = _split_drain_and_barrier


def _split_excess_waits(nc):
    """Hoist excess per-instruction sync waits into standalone EVSEM waits.

    InstDmaTransposeAnt cannot encode ANY sync wait (walrus codegen
    "Too many sync wait commands"), so all of its waits move onto
    EVSEMs before it; other instructions keep 1 wait (EVSEM: 2)."""
    counter = 0
    for f in nc.m.functions:
        for bb in f.blocks:
            insts = bb.instructions
            out = []
            for inst in insts:
                si = inst.sync_info
                tname = type(inst).__name__
                if "DmaTranspose" in tname:
                    cap = 0
                elif isinstance(inst, mybir.InstEventSemaphore):
                    cap = 2
                else:
                    cap = 1
                if si is not None and si.on_wait and len(si.on_wait) > cap:
                    waits = list(si.on_wait)
                    for w in waits[cap:]:
                        counter += 1
                        ev = mybir.InstEventSemaphore(name=f"I-wsplit-{counter}")
                        ev.engine = inst.engine
                        ev.sync_info = mybir.SyncInfo(on_wait=[w], on_update=[])
                        out.append(ev)
                    si.on_wait = waits[:cap]
                out.append(inst)
            if len(out) != len(insts):
                insts[:] = out
    return counter


P = 128
S = 1024
E = 768
H = 12
D = 64
KT = E // P        # 6 e-tiles
ST = S // P        # 8 s-tiles
NPAIR = H // 2     # 6 head pairs
QTILE = 512
NQ = S // QTILE    # 2 q-tiles
ESLICES = [(0, 512), (512, 256)]

f32 = mybir.dt.float32
f16 = mybir.dt.float16
bf16 = mybir.dt.bfloat16
EXP = mybir.ActivationFunctionType.Exp
LN = mybir.ActivationFunctionType.Ln

_NC_CACHE = {}


def build(mm_dtype="f16", e_dtype="f16"):
    mdt = {"f16": f16, "bf16": bf16}[mm_dtype]
    edt = {"f16": f16, "bf16": bf16}[e_dtype]
    nc = bass.Bass()
    x_d = nc.declare_dram_parameter("x", [S, E], f32, isOutput=False)
    Wq_d = nc.declare_dram_parameter("Wq", [E, E], f32, isOutput=False)
    Wk_d = nc.declare_dram_parameter("Wk", [E, E], f32, isOutput=False)
    Wv_d = nc.declare_dram_parameter("Wv", [E, E], f32, isOutput=False)
    Wo_d = nc.declare_dram_parameter("Wo", [E, E], f32, isOutput=False)
    bq_d = nc.declare_dram_parameter("bq", [E], f32, isOutput=False)
    bk_d = nc.declare_dram_parameter("bk", [E], f32, isOutput=False)
    bv_d = nc.declare_dram_parameter("bv", [E], f32, isOutput=False)
    bo_d = nc.declare_dram_parameter("bo", [E], f32, isOutput=False)
    out_d = nc.declare_dram_parameter("out", [S, E], f32, isOutput=True)

    with ExitStack() as ctx:
        tc = ctx.enter_context(tile.TileContext(nc))
        singles = ctx.enter_context(tc.tile_pool(name="singles", bufs=1))
        xld = ctx.enter_context(tc.tile_pool(name="xld", bufs=8))
        x16p = ctx.enter_context(tc.tile_pool(name="x16p", bufs=8))
        wst = ctx.enter_context(tc.tile_pool(name="wst", bufs=3))
        wqk = ctx.enter_context(tc.tile_pool(name="wqk", bufs=2))
        wbig = ctx.enter_context(tc.tile_pool(name="wbig", bufs=1))
        qkp = ctx.enter_context(tc.tile_pool(name="qkp", bufs=2))
        ep = ctx.enter_context(tc.tile_pool(name="ep", bufs=2))
        np_pool = ctx.enter_context(tc.tile_pool(name="norm", bufs=2))
        outp = ctx.enter_context(tc.tile_pool(name="outp", bufs=2))
        bcast = ctx.enter_context(tc.tile_pool(name="bcast", bufs=1))
        # PSUM: S ([P,2,512]x2 = 4 banks) + mm ([P,512]x2) + att ([P,512]x2)
        psum = ctx.enter_context(tc.tile_pool(name="psum", bufs=2, space="PSUM"))

        # ---- persistent big buffers ----
        xT = singles.tile([P, KT, S], mdt)          # x^T  [e_in, s]
        V_ext = singles.tile([P, ST, H, P], edt)    # [s, ktile, head, ...]
        concatT = singles.tile([P, NPAIR, S], mdt)  # attn^T by pair

        # ---- phase 0: DMAs + casts + XBAR transposes ----
        # sync queue: x tiles (+ b vectors); gpsimd: Wv staging, bcasts,
        # V_ext ones-memsets; DVE: all f32->f16 casts; scalar: transposes.
        x_sb, x16 = {}, {}
        for st in range(ST):
            x_sb[st] = xld.tile([P, E], f32, tag="x", name="x_sb")
            x16[st] = x16p.tile([P, E], mdt, tag="x16", name="x16")

        Wv_sb = wbig.tile([P, KT, E], mdt, tag="wbig")
        Wv_re = Wv_d[:].rearrange("(ko p) m -> p ko m", p=P)
        wv_stg = []
        for j in range(KT):
            wv_stg.append(wst.tile([P, E], f32, tag="wstage", name="wstage"))

        def bcast_load(dst, src_ap):  # [E] -> [P, E] partition-step-0 DMA
            nc.gpsimd.dma_start(
                out=dst,
                in_=bass.AP(tensor=src_ap.tensor, offset=src_ap.offset,
                            ap=[[0, P]] + [list(a) for a in src_ap.ap]))

        bv_bc = bcast.tile([P, E], f32, tag="bvbc")
        bo_bc = bcast.tile([P, E], f32, tag="bobc")
        v4 = V_ext[:].rearrange("p st (hh two) d -> p st hh two d", two=2)

        # sync queue: x DMAs in order
        for st in range(ST):
            nc.sync.dma_start(x_sb[st][:], x_d[st * P:(st + 1) * P, :])
        # gpsimd queue: Wv staging interleaved with bcasts + ones-memsets
        nc.gpsimd.dma_start(wv_stg[0][:], Wv_re[:, 0, :])
        nc.gpsimd.dma_start(wv_stg[1][:], Wv_re[:, 1, :])
        bcast_load(bv_bc[:], bv_d[:])
        nc.gpsimd.memset(v4[:, 0, :, 0, D:P], 1.0)
        nc.gpsimd.memset(v4[:, 0, :, 1, 0:D], 1.0)
        for j in range(2, KT):
            nc.gpsimd.dma_start(wv_stg[j][:], Wv_re[:, j, :])
            st = j - 1
            nc.gpsimd.memset(v4[:, st, :, 0, D:P], 1.0)
            nc.gpsimd.memset(v4[:, st, :, 1, 0:D], 1.0)
        for st in range(KT - 1, ST):
            nc.gpsimd.memset(v4[:, st, :, 0, D:P], 1.0)
            nc.gpsimd.memset(v4[:, st, :, 1, 0:D], 1.0)
        bcast_load(bo_bc[:], bo_d[:])
        # DVE: casts — x tiles lead (they feed the XBAR transposes), Wv
        # chunks interleave so vproj's k-loop is fed in order.
        dve_order = [("x", 0), ("x", 1), ("w", 0), ("x", 2), ("w", 1),
                     ("x", 3), ("w", 2), ("x", 4), ("w", 3), ("x", 5),
                     ("w", 4), ("x", 6), ("w", 5), ("x", 7)]
        for kind, i in dve_order:
            if kind == "x":
                nc.vector.tensor_copy(x16[i][:], x_sb[i][:])
            else:
                nc.vector.tensor_copy(Wv_sb[:, i, :], wv_stg[i][:])
        # scalar queue: XBAR transposes into xT
        for st in range(ST):
            nc.scalar.dma_start_transpose(
                xT[:, :, st * P:(st + 1) * P], x16[st][:])
        # small bias loads (sync, after x)
        bq_sb = singles.tile([P, KT], f32)
        bk_sb = singles.tile([P, KT], f32)
        nc.sync.dma_start(bq_sb[:], bq_d[:].rearrange("(o p) -> p o", p=P))
        nc.sync.dma_start(bk_sb[:], bk_d[:].rearrange("(o p) -> p o", p=P))

        # ---- phase 2: V projection per s-tile ----
        def vproj_st(st):
            pv = psum.tile([P, 2, 512], f32, tag="S", name="pv")
            for k in range(KT):  # k-outer: xT stationary reused across nsi
                for nsi, (noff, nsz) in enumerate(ESLICES):
                    nc.tensor.matmul(
                        pv[:, nsi, :nsz],
                        xT[:, k, st * P:(st + 1) * P],
                        Wv_sb[:, k, noff:noff + nsz],
                        start=(k == 0), stop=(k == KT - 1),
                    )
            # batched scatter: evens -> [V|ones] cols 0:64, odds -> 64:128
            for nsi, (noff, nsz) in enumerate(ESLICES):
                nh = nsz // P
                hh0 = 4 * nsi
                pvr = pv[:, nsi, :nsz].rearrange(
                    "p (hh two d) -> p hh two d", two=2, d=D)
                bvr = bv_bc[:, noff:noff + nsz].rearrange(
                    "p (hh two d) -> p hh two d", two=2, d=D)
                nc.vector.tensor_add(
                    v4[:, st, hh0:hh0 + nh, 0, 0:D], pvr[:, :, 0, :],
                    bvr[:, :, 0, :])
                nc.vector.tensor_add(
                    v4[:, st, hh0:hh0 + nh, 1, D:P], pvr[:, :, 1, :],
                    bvr[:, :, 1, :])

        for st in range(ST):
            vproj_st(st)

        # ---- phase 3: head pairs, software-pipelined ----
        wq_t, wk_t, qt_t, kt_t = {}, {}, {}, {}
        Wq_re = Wq_d[:].rearrange("(ko p) m -> p ko m", p=P)
        Wk_re = Wk_d[:].rearrange("(ko p) m -> p ko m", p=P)

        def load_w(m):
            wq_t[m] = wqk.tile([P, KT, P], mdt, tag="wq", name="wq_m")
            wk_t[m] = wqk.tile([P, KT, P], mdt, tag="wk", name="wk_m")
            for which, dst, src in (("q", wq_t[m], Wq_re), ("k", wk_t[m], Wk_re)):
                stg = wst.tile([P, KT, P], f32, tag="wqs" + which, name="wqs")
                nc.sync.dma_start(stg[:], src[:, :, m * P:(m + 1) * P])
                nc.vector.tensor_copy(dst[:], stg[:])

        def proj_one(m, which):
            """12 matmuls: full QT_m (or KT_m) over both q-halves."""
            w = wq_t[m] if which == "q" else wk_t[m]
            bias = bq_sb if which == "q" else bk_sb
            t = qkp.tile([P, S], mdt, tag=which + "t", name=which + "t")
            (qt_t if which == "q" else kt_t)[m] = t
            for q2 in range(NQ):
                qsl = slice(q2 * QTILE, (q2 + 1) * QTILE)
                pq = psum.tile([P, 512], f32, tag="mm", name="pq")
                for k in range(KT):
                    nc.tensor.matmul(pq[:], w[:, k, :], xT[:, k, qsl],
                                     start=(k == 0), stop=(k == KT - 1))
                nc.vector.tensor_scalar_add(t[:, qsl], pq[:], bias[:, m:m + 1])

        def wload_big(dst16, src_re):
            for j in range(KT):
                stg = wst.tile([P, E], f32, tag="wstage", name="wstage")
                nc.sync.dma_start(stg[:], src_re[:, j, :])
                nc.vector.tensor_copy(dst16[:, j, :], stg[:])

        def outproj_st(st):
            o_sb = outp.tile([P, E], f32, tag="o")
            for nsi, (noff, nsz) in enumerate(ESLICES):
                po = psum.tile([P, 512], f32, tag="mm", name="po")
                for k in range(KT):
                    nc.tensor.matmul(
                        po[:, :nsz],
                        concatT[:, k, st * P:(st + 1) * P],
                        Wo_sb[:, k, noff:noff + nsz],
                        start=(k == 0), stop=(k == KT - 1),
                    )
                nc.vector.tensor_add(o_sb[:, noff:noff + nsz], po[:, :nsz],
                                     bo_bc[:, noff:noff + nsz])
            nc.sync.dma_start(out_d[st * P:(st + 1) * P, :], o_sb[:])

        load_w(0)
        proj_one(0, "q")
        proj_one(0, "k")
        # Wo loaded+cast here: overlaps the attention phase; the wbig slot
        # becomes free once the last V-proj matmul has read Wv.
        Wo_sb = wbig.tile([P, KT, E], mdt, tag="wbig")
        wload_big(Wo_sb, Wo_d[:].rearrange("(ko p) m -> p ko m", p=P))
        for m in range(NPAIR):
            if m + 1 < NPAIR:
                load_w(m + 1)
            qt_m, kt_m = qt_t[m], kt_t[m]
            for q2 in range(NQ):
                qsl = slice(q2 * QTILE, (q2 + 1) * QTILE)
                # proj for the next pair FIRST: independent PE work in front
                # of the scores c-loop, which is paced by ACT draining the
                # previous unit's exps out of the S-ring.
                if m + 1 < NPAIR:
                    proj_one(m + 1, "q" if q2 == 0 else "k")
                e_a = ep.tile([P, ST, QTILE], edt, tag="eA")
                e_b = ep.tile([P, ST, QTILE], edt, tag="eB")
                for c in range(ST // 2):
                    s_a = psum.tile([P, 2, 512], f32, tag="S", name="s_a")
                    s_b = psum.tile([P, 2, 512], f32, tag="S", name="s_b")
                    for kk in range(2):
                        ktile = c * 2 + kk
                        ksl = slice(ktile * P, (ktile + 1) * P)
                        nc.tensor.matmul(s_a[:, kk, :], kt_m[0:D, ksl],
                                         qt_m[0:D, qsl], start=True, stop=True)
                        nc.tensor.matmul(s_b[:, kk, :], kt_m[D:P, ksl],
                                         qt_m[D:P, qsl], start=True, stop=True)
                    nc.scalar.activation(e_a[:, c * 2:c * 2 + 2, :], s_a[:], EXP, scale=0.125)
                    nc.scalar.activation(e_b[:, c * 2:c * 2 + 2, :], s_b[:], EXP, scale=0.125)
                # outproj s-tiles 0-3 need only q-half 0 of concatT: slot
                # their matmuls between the last unit's scores (which feed
                # ACT) and its attnV (which waits on those exps).
                if m == NPAIR - 1 and q2 == 1:
                    for st in range(4):
                        outproj_st(st)
                # attnV: rows [attn|sums] (even head) / [sums|attn] (odd head)
                p_a = psum.tile([P, 512], f32, tag="att", name="p_a")
                p_b = psum.tile([P, 512], f32, tag="att", name="p_b")
                for ktile in range(ST):
                    nc.tensor.matmul(p_a[:], V_ext[:, ktile, 2 * m, :],
                                     e_a[:, ktile, :],
                                     start=(ktile == 0), stop=(ktile == ST - 1))
                for ktile in range(ST):
                    nc.tensor.matmul(p_b[:], V_ext[:, ktile, 2 * m + 1, :],
                                     e_b[:, ktile, :],
                                     start=(ktile == 0), stop=(ktile == ST - 1))
                # Normalize straight from PSUM with partition-aligned reads.
                # 1/sums via a bit-trick seed + one Newton step on plain DVE
                # ops.  (ACT Ln/Exp would be simpler but using Ln anywhere
                # forces walrus onto the combined natural_log_exp table,
                # which slows EVERY exp ~15% — measured 1116 -> 1281ns —
                # and ACT is the phase-3 co-bottleneck.)  seed bits =
                # ~(s_bits + ~K) = K - s_bits, rel err ~5%; r1 = r0*(2-s*r0)
                # lands at ~2.6e-3, below the fp16 concatT quantization.
                # The half-swap runs on the idle gpsimd queue: DVE cannot
                # read SBUF/PSUM at a partition base different from its
                # output's (verifier NCC_IBIR297 / measured corruption).
                rec_t = np_pool.tile([P, 512], f32, tag="rec_t")
                tnew = np_pool.tile([P, 512], f32, tag="tnew")
                nrec = np_pool.tile([P, 512], f32, tag="nrec")
                rec = np_pool.tile([P, 512], f32, tag="rec")
                NOT_K = ~0x7EF311C2
                i32 = mybir.dt.int32
                AO = mybir.AluOpType
                nc.vector.tensor_scalar(
                    rec_t[D:P, :].bitcast(i32), p_a[D:P, :].bitcast(i32),
                    NOT_K, None, op0=AO.add)
                nc.vector.tensor_scalar(
                    rec_t[0:D, :].bitcast(i32), p_b[0:D, :].bitcast(i32),
                    NOT_K, None, op0=AO.add)
                nc.vector.tensor_scalar(
                    nrec[:].bitcast(i32), rec_t[:].bitcast(i32),
                    -1, None, op0=AO.bitwise_xor)
                nc.vector.tensor_mul(tnew[D:P, :], p_a[D:P, :], nrec[D:P, :])
                nc.vector.tensor_mul(tnew[0:D, :], p_b[0:D, :], nrec[0:D, :])
                # rec_t = (t - 2) * r0 = -r1
                nc.vector.scalar_tensor_tensor(
                    rec_t[:], tnew[:], 2.0, nrec[:], op0=AO.subtract, op1=AO.mult)
                nc.gpsimd.dma_start(rec[0:D, :], rec_t[D:P, :])
                nc.gpsimd.dma_start(rec[D:P, :], rec_t[0:D, :])
                # (-p) * (-r1) = p/sums
                nc.vector.scalar_tensor_tensor(
                    concatT[0:D, m, qsl], p_a[0:D, :], -1.0, rec[0:D, :],
                    op0=AO.mult, op1=AO.mult)
                nc.vector.scalar_tensor_tensor(
                    concatT[D:P, m, qsl], p_b[D:P, :], -1.0, rec[D:P, :],
                    op0=AO.mult, op1=AO.mult)

        # ---- phase 4: output projection, remaining s-tiles ----
        for st in range(4, ST):
            outproj_st(st)

    _split_excess_waits(nc)
    return nc


def run_spmd(inputs, Wq, bq, Wk, bk, Wv, bv, Wo, bo,
             mm_dtype="f16", e_dtype="f16", trace=False):
    key = (mm_dtype, e_dtype)
    if key not in _NC_CACHE:
        _NC_CACHE[key] = build(mm_dtype, e_dtype)
    nc = _NC_CACHE[key]
    x = np.asarray(inputs, dtype=np.float32)
    common = {
        "Wq": np.asarray(Wq, np.float32), "Wk": np.asarray(Wk, np.float32),
        "Wv": np.asarray(Wv, np.float32), "Wo": np.asarray(Wo, np.float32),
        "bq": np.asarray(bq, np.float32), "bk": np.asarray(bk, np.float32),
        "bv": np.asarray(bv, np.float32), "bo": np.asarray(bo, np.float32),
    }
    in_maps = [dict(common, x=np.ascontiguousarray(x[b])) for b in range(x.shape[0])]
    res = run_bass_kernel_spmd(nc, in_maps, core_ids=list(range(len(in_maps))),
                               trace=trace)
    out = np.stack([res.results[b]["out"] for b in range(len(in_maps))], axis=0)
    return out, res


def kernel(inputs, Wq, bq, Wk, bk, Wv, bv, Wo, bo):
    out, _ = run_spmd(inputs, Wq, bq, Wk, bk, Wv, bv, Wo, bo)
    return out
